# revision 45
# baseline (speedup 1.0000x reference)
"""Weighted-BCE loss kernel for Trainium2 (8 NeuronCores, SPMD data-parallel).

Reference math (torch-style BCELoss with class-balancing weights):
    n   = len(x), s = sum(gt)
    w0  = n / (2*(n-s)),  w1 = n / (2*s)
    L1  = max(log(x),     -100)
    L0  = max(log1p(-x),  -100)
    loss = mean( where(gt==0, w0, w1) * -(gt*L1 + (1-gt)*L0) )

The weights depend only on the GLOBAL positive count s, so the loss
decomposes into 4 global sums computed shard-locally:
    A = sum(gt * L1),  B = sum(gt * L0u),  C = sum(L0u),  s = sum(gt)
    loss = -( A/(2s) + (C-B)/(2(n-s)) )
L0u is UNclamped log(1-x): x is fp32 in [0,1), so 1-x >= 2^-25 and
log(1-x) >= -17.4 — the -100 clamp can never fire on the L0 branch.
The L1 clamp IS needed (x == 0 -> ACT Ln gives -inf, measured) and rides
for free inside the DVE's fused scalar_tensor_tensor op.

Engine split per 1/8 shard (2M elements as [128 partitions, 16384 free]):
  - gt is narrowed to bf16 on the host (0/1 — exact): 2/3 the DMA bytes,
    PE-compatible dtype, and 2x-mode DVE operand.
  - ScalarE (ACT) is the critical engine: exactly two Ln passes per tile
    (ACT is 1x rate at every dtype, measured), writing bf16, no accum.
  - VectorE (DVE):
      A += reduce((lnx max -100) * gt)   fused scalar_tensor_tensor
      prodB = gt * ln1                   tensor_tensor, 2x bf16 mode
    A's short cascade (ACT->DVE->out) keeps the kernel tail small.
  - TensorE (PE, otherwise idle) reduces the rest against a fixed
    ones[128,1] stationary at full rate (512 cols / 216ns), each sum
    accumulating into its own PSUM bank across all tiles (a bank is
    2KB/partition = 512 f32; the matmul start-bit resets a whole bank):
      S += ones.T @ gt_chunk     C += ones.T @ ln1_chunk
      B += ones.T @ prodB_chunk
  - All input DMA on the sync HWDGE ring, per-tile [x_i, gt_i] waves
    (x first: ACT is the longest chain; gt right behind for the DVE);
    compute engines issue no DMAs; bufs=4/4 on x/gt keeps descriptors
    always eligible so the ring runs at the HBM roofline (~410 GB/s).
  - First/last tiles are sized to shrink pipeline ramp and drain.
Host gathers accA + the [1, 1536] psum rows from all 8 cores and
finishes the (tiny) all-reduce + final scalar arithmetic in float64.
"""

import numpy as np
import ml_dtypes
from contextlib import ExitStack

import concourse.bass as bass
import concourse.bacc as bacc
import concourse.mybir as mybir
import concourse.tile as tile
from concourse.alu_op_type import AluOpType
from concourse.bass_utils import run_bass_kernel_spmd

N_TOTAL = 16777216
N_CORES = 8
PER_CORE = N_TOTAL // N_CORES   # 2097152
P = 128
FD = PER_CORE // P              # 16384 free elements per partition
TILE_SIZES = [2048, 3072, 4096, 4608, 2048, 512]   # all multiples of 512
assert sum(TILE_SIZES) == FD
NT = len(TILE_SIZES)
LOG_CLAMP = -100.0

# Optional instrumentation knobs for a driver script (harness never sets them).
TRACE = False
LAST_RESULTS = None

_NC_CACHE = None


def _build():
    f32 = mybir.dt.float32
    bf16 = mybir.dt.bfloat16
    Ln = mybir.ActivationFunctionType.Ln

    nc = bacc.Bacc("TRN2")
    x_in = nc.declare_dram_parameter("x", [P, FD], f32, isOutput=False)
    g_in = nc.declare_dram_parameter("gt", [P, FD], bf16, isOutput=False)
    outA = nc.declare_dram_parameter("outA", [P, NT], f32, isOutput=True)
    # [B | C | S] partial rows, 512 columns each, packed side by side
    outBCS = nc.declare_dram_parameter("outBCS", [1, 1536], f32, isOutput=True)

    with tile.TileContext(nc) as tc, ExitStack() as ctx:
        xp = ctx.enter_context(tc.tile_pool(name="xp", bufs=4))
        gp = ctx.enter_context(tc.tile_pool(name="gp", bufs=4))
        lp = ctx.enter_context(tc.tile_pool(name="lp", bufs=2))
        prp = ctx.enter_context(tc.tile_pool(name="prp", bufs=2))
        jp = ctx.enter_context(tc.tile_pool(name="jp", bufs=1))
        accp = ctx.enter_context(tc.tile_pool(name="accp", bufs=1))
        pp = ctx.enter_context(tc.psum_pool(name="pp", bufs=1))

        accA = accp.tile([P, NT], f32)
        ones = accp.tile([P, 1], bf16)
        nc.gpsimd.memset(ones[:], 1.0)

        # one psum tile spanning 3 banks: B, C, S (one bank each)
        psBCS = pp.tile([1, 3 * 512], f32)
        psB = psBCS[:, 0:512]
        psC = psBCS[:, 512:1024]
        psS = psBCS[:, 1024:1536]

        ns_total = FD // 512
        done = {"B": 0, "C": 0, "S": 0}

        def reduce_chunks(ps, key, src, tfd):
            for c in range(tfd // 512):
                cs = slice(c * 512, (c + 1) * 512)
                nc.tensor.matmul(ps, ones[:], src[:, cs],
                                 start=(done[key] == 0),
                                 stop=(done[key] == ns_total - 1))
                done[key] += 1

        off = 0
        for i, tfd in enumerate(TILE_SIZES):
            sl = slice(off, off + tfd)
            off += tfd
            xt = xp.tile([P, tfd], f32, tag="xt")
            gt_t = gp.tile([P, tfd], bf16, tag="gt")
            # single HWDGE ring (sync): x first — ACT is the longest chain
            nc.sync.dma_start(xt[:], x_in[:, sl])
            nc.sync.dma_start(gt_t[:], g_in[:, sl])

            # S += column sums of gt (PE)
            reduce_chunks(psS, "S", gt_t, tfd)

            # pass 1: ln(x); A's fused clamp+mult+reduce goes on DVE
            lnx = lp.tile([P, tfd], bf16, tag="lnx")
            nc.scalar.activation(lnx[:], xt[:], Ln)
            junk = jp.tile([P, tfd], bf16, tag="junk")
            nc.vector.scalar_tensor_tensor(
                junk[:], lnx[:], LOG_CLAMP, gt_t[:],
                AluOpType.max, AluOpType.mult,
                accum_out=accA[:, i : i + 1],
            )

            # pass 2: ln(1-x); B = sum(gt*ln1) via DVE product + PE reduce
            ln1 = lp.tile([P, tfd], bf16, tag="ln1")
            nc.scalar.activation(ln1[:], xt[:], Ln, bias=1.0, scale=-1.0)
            prodB = prp.tile([P, tfd], bf16, tag="prodB")
            nc.vector.tensor_tensor(prodB[:], gt_t[:], ln1[:], AluOpType.mult)
            reduce_chunks(psB, "B", prodB, tfd)
            reduce_chunks(psC, "C", ln1, tfd)

        # drain accumulated psums to SBUF, then DRAM
        sbBCS = accp.tile([1, 3 * 512], f32)
        nc.scalar.copy(sbBCS[:], psBCS[:])
        nc.sync.dma_start(outBCS[:], sbBCS[:])
        nc.sync.dma_start(outA[:], accA[:])

    nc.compile()
    return nc


def get_nc():
    global _NC_CACHE
    if _NC_CACHE is None:
        _NC_CACHE = _build()
    return _NC_CACHE


def make_in_maps(x, gt):
    x = np.ascontiguousarray(np.asarray(x, dtype=np.float32).reshape(-1))
    gt = np.asarray(gt).reshape(-1)
    assert x.shape == (N_TOTAL,) and gt.shape == (N_TOTAL,)
    # narrow the 0/1 labels to bf16 (exact): 2/3 the DMA bytes, PE-compatible
    gtb = np.ascontiguousarray(gt.astype(ml_dtypes.bfloat16))
    in_maps = []
    for c in range(N_CORES):
        sl = slice(c * PER_CORE, (c + 1) * PER_CORE)
        in_maps.append({
            "x": x[sl].reshape(P, FD),
            "gt": gtb[sl].reshape(P, FD),
        })
    return in_maps


def combine(results):
    """All-reduce the per-core partial sums and finish the loss formula."""
    A = B = C = S = 0.0
    for r in results:
        A += r["outA"].astype(np.float64).sum()
        bcs = r["outBCS"].astype(np.float64).reshape(3, 512)
        B += bcs[0].sum()
        C += bcs[1].sum()
        S += bcs[2].sum()
    n = float(N_TOTAL)
    result = -(A / (2.0 * S) + (C - B) / (2.0 * (n - S)))
    return np.array(result, dtype=np.float32)


def kernel(x, gt):
    global LAST_RESULTS
    nc = get_nc()
    in_maps = make_in_maps(x, gt)
    br = run_bass_kernel_spmd(nc, in_maps, list(range(N_CORES)))
    LAST_RESULTS = br
    return combine(br.results)
